# revision 49
# baseline (speedup 1.0000x reference)
"""Sparse attention (RoPE'd Q=K, strictly-causal unnormalized scores @ V).

  Q: (1, 4, 2048, 8192) f32   V: (1, 1, 2048, 256) f32
  out = tril(QR @ QR^T, -1) @ V   per head, V broadcast over heads.

Sharding: 8 cores = 4 heads x 2 halves of the N=8192 contraction dim.
The causal mask is elementwise, so masked-scores @ V is additive over
N-slices: each core computes a full (2048, 256) partial output from its
(2048, 4096) slice of QR; host sums the two halves per head.

Device algorithm (chunked linear attention, chunk C=256):
  out[t] = QR[t] @ S_{<chunk} + (intra-chunk causal part), where
  S = sum_s QR[s] (x) V[s] is an [N_c, D] state accumulated chunk by chunk.

v2 over the f32r baseline:
  - All matmul operands bf16 (rel err ~4e-3 vs 2e-2 budget): halves HBM
    traffic and lifts the f32r free-dim>=256 restriction, so the dead
    lower-left intra block is skipped (st1 computed only for its live
    128 columns). PE cost model: 1 cycle/row at any free size.
  - State kept as 16 pairs [128, 2*D] bf16; each update accumulates into
    one PSUM bank [128, 512] f32 and is folded with a single paired
    tensor_add, split DVE (first 9, matches Q@S consumption order) /
    GPSIMD (last 7).
  - PSUM->SBUF evictions (out rows, intra cross block) on the otherwise
    idle Activation engine.

Host does RoPE in f32, then packs bf16 so each chunk is a single large
DMA with 8 KiB contiguous descriptor runs per partition:
  qrt_p[c]  = [128, 32*256]  (SBUF layout: partition p=n%128, k-tile major)
  qtn_p[c]  = [128, 2*4096]  (partition p=t%128 within chunk)
  v_p       = [128, 16*256]
"""

import math

import numpy as np

THETA = 2.0**16
TWO_PI = 2.0 * math.pi

B, NH, T, N, D = 1, 4, 2048, 8192, 256
NSPLIT = 2
NCORES = NH * NSPLIT
NC_FEAT = N // NSPLIT  # 4096 features per core
P = 128
KT = NC_FEAT // P  # 32 n-tiles
KP = KT // 2  # 16 state pairs
TT = T // P  # 16 t-tiles
C = 256  # chunk length
NCH = T // C  # 8 chunks
CSUB = C // P  # 2 t-subtiles per chunk
# qrt DMA piece size in k-tiles: piece 0's matmul work covers piece 1's
# arrival, so 4 uniform pieces stream without quantization stalls
QPIECES = (8, 8, 8, 8)
VSPLIT = 6  # v subtiles loaded up front (covers chunks 0-2); rest deferred

_COMPILED = None
_ROPE_E = None


def _rope_tables():
    """cos/sin as one complex table; frequencies are pair-constant, so only
    even columns are needed. Input-independent -> cached across calls."""
    global _ROPE_E
    if _ROPE_E is None:
        idx = (np.floor(np.arange(N, dtype=np.float32) / 2.0) * 2.0).astype(
            np.float32
        )
        freqs = (1.0 / (THETA ** (idx / np.float32(N))) / np.float32(TWO_PI)).astype(
            np.float32
        )
        t = np.arange(T, dtype=np.float32)
        phases = t[:, None] * freqs[None, ::2]
        ang = np.float32(TWO_PI) * (phases % np.float32(1.0))
        E = np.empty((T, N // 2), np.complex64)
        E.real = np.cos(ang)
        E.imag = np.sin(ang)
        _ROPE_E = E
    return _ROPE_E


def _rope_host(Q):
    """(a+bi)(c+si) = (ac-bs) + (as+bc)i == the reference's interleaved
    rotate-pairs RoPE, one pass over Q viewed as complex64."""
    E = _rope_tables()
    QRc = Q.view(np.complex64) * E
    return QRc.view(np.float32)


def _mask_host():
    """mask[si, tj] = 1 if si < tj; shared by both diagonal intra blocks."""
    si = np.arange(P)[:, None]
    tj = np.arange(P)[None, :]
    return (si < tj).astype(np.float32)  # [128, 128]


def _build():
    import concourse.tile as tile
    from concourse import bacc, mybir

    nc = bacc.Bacc(
        "TRN2",
        target_bir_lowering=False,
        debug=False,
        enable_asserts=False,
        num_devices=NCORES,
    )
    f32 = mybir.dt.float32
    bf16 = mybir.dt.bfloat16

    qrt = nc.dram_tensor("qrt", [NCH, P, KT * C], bf16, kind="ExternalInput").ap()
    qtn = nc.dram_tensor("qtn", [NCH, P, KT * C], bf16, kind="ExternalInput").ap()
    v = nc.dram_tensor("v", [P, TT * D], bf16, kind="ExternalInput").ap()
    mask = nc.dram_tensor("mask", [P, P], f32, kind="ExternalInput").ap()
    out = nc.dram_tensor("out", [T, D], bf16, kind="ExternalOutput").ap()

    with tile.TileContext(nc) as tc:
        with (
            tc.tile_pool(name="qr", bufs=12) as qp,
            tc.tile_pool(name="qt", bufs=9) as tp,
            tc.tile_pool(name="vp", bufs=1) as vp,
            tc.tile_pool(name="mk", bufs=1) as mp,
            tc.tile_pool(name="st", bufs=KP) as stp,
            tc.tile_pool(name="sc", bufs=4) as sp,
            tc.tile_pool(name="ob", bufs=3) as op_,
            tc.tile_pool(name="tm", bufs=3) as tmp_,
            tc.tile_pool(name="p0", bufs=1, space="PSUM") as pp0,
            tc.tile_pool(name="po", bufs=2, space="PSUM") as ppo,
            tc.tile_pool(name="pu", bufs=5, space="PSUM") as ppu,
        ):
            vtiles = None
            mtile = None
            # state pair j holds S[2j] | S[2j+1], each [128, D]
            Spairs = [
                stp.tile([P, 2 * D], bf16, tag="S", name=f"S{j}") for j in range(KP)
            ]

            # warm the Activation func table during the startup DMA wait so
            # the implicit LoadActFuncSet is off the critical path
            warm = tmp_.tile([P, 1], f32, tag="wu", name="warm")
            nc.vector.memset(warm, 0.0)
            nc.scalar.copy(warm, warm)
            # burn the PE pstate ramp on garbage matmuls while the first qrt
            # piece is in flight: by the first real matmul the clock is at
            # 2.4GHz instead of spending chunk 0 at 0.65-1.2GHz
            wb = tmp_.tile([P, 2 * P], bf16, tag="wb", name="wb")
            nc.vector.memset(wb, 0.0)
            wpo = ppo.tile([P, 2 * P], f32, tag="po", name="warm_po")
            for i in range(15):
                nc.tensor.matmul(
                    wpo, lhsT=wb[:, 0:P], rhs=wb, start=(i == 0), stop=(i == 14)
                )

            def S_k(k):
                return Spairs[k // 2][:, (k % 2) * D : (k % 2) * D + D]

            for c in range(NCH):
                c0 = c * C
                pieces = QPIECES
                qh = []  # (first_ktile, tile)
                k0 = 0
                for u, nk in enumerate(pieces):
                    qhu = qp.tile([P, nk * C], bf16, tag="qr", name=f"q{c}_{u}")
                    nc.sync.dma_start(
                        out=qhu, in_=qrt[c][:, k0 * C : (k0 + nk) * C]
                    )
                    qh.append((k0, qhu))
                    k0 += nk
                    if c == 0 and u == 0:
                        # tiny; lands before the first st mask-mul needs it
                        mtile = mp.tile([P, P], f32)
                        nc.sync.dma_start(out=mtile, in_=mask)

                def qslice(k, lo, hi):
                    for k0, qhu in reversed(qh):
                        if k >= k0:
                            return qhu[:, (k - k0) * C + lo : (k - k0) * C + hi]
                    raise AssertionError

                if c == 0:
                    # v split: the early phase is bus-bound (qrt_0+qtn_0+qrt_1
                    # must land before chunk 1), so defer most of v past qrt_1
                    vt = vp.tile([P, TT * D], bf16)
                    nc.sync.dma_start(
                        out=vt[:, : VSPLIT * D], in_=v[:, : VSPLIT * D]
                    )
                    vtiles = [vt[:, a * D : (a + 1) * D] for a in range(TT)]
                if c == 1:
                    nc.sync.dma_start(
                        out=vt[:, VSPLIT * D :], in_=v[:, VSPLIT * D :]
                    )

                # qtn is packed k-major ([k, m, n] per partition row), so the
                # update can start after the first piece instead of the full
                # 2 MB (the early chunks are DMA-bandwidth-bound)
                tn_pieces = []  # (first_ktile, tile)
                if c < NCH - 1:
                    tk0 = 0
                    for nk in QPIECES:
                        tnp = tp.tile(
                            [P, nk * C], bf16, tag="tn", name=f"tn{c}_{tk0}"
                        )
                        nc.sync.dma_start(
                            out=tnp, in_=qtn[c][:, tk0 * C : (tk0 + nk) * C]
                        )
                        tn_pieces.append((tk0, tnp))
                        tk0 += nk

                def tnslice(k, m):
                    for tk0, tnp in reversed(tn_pieces):
                        if k >= tk0:
                            base = (k - tk0) * C + m * P
                            return tnp[:, base : base + P]
                    raise AssertionError

                # intra-chunk causal scores, [s, t] upper layout.
                # Block s0 x (t0|t1): [128, 256]; block s1 x t1: [128, 128]
                # (s1 x t0 is identically zero and skipped).
                pi_t = pp0.tile([P, C + P], f32, tag="ps", name=f"ps_{c}")
                ps0 = pi_t[:, 0:C]
                ps1 = pi_t[:, C : C + P]
                if c == 0:
                    # chunk 0 streams behind its own DMA: interleave both
                    # score groups per k so each arriving piece carries 2x
                    # the matmul work. Interleaved open accumulation groups
                    # must sit in DIFFERENT banks: borrow a po slot for ps1.
                    ps1 = ppo.tile([P, P], f32, tag="po", name="ps1_0")
                    for k in range(KT):
                        nc.tensor.matmul(
                            ps0,
                            lhsT=qslice(k, 0, P),
                            rhs=qslice(k, 0, C),
                            start=(k == 0),
                            stop=(k == KT - 1),
                            skip_group_check=True,
                        )
                        nc.tensor.matmul(
                            ps1,
                            lhsT=qslice(k, P, C),
                            rhs=qslice(k, P, C),
                            start=(k == 0),
                            stop=(k == KT - 1),
                            skip_group_check=True,
                        )
                else:
                    for k in range(KT):
                        nc.tensor.matmul(
                            ps0,
                            lhsT=qslice(k, 0, P),
                            rhs=qslice(k, 0, C),
                            start=(k == 0),
                            stop=(k == KT - 1),
                        )
                    for k in range(KT):
                        nc.tensor.matmul(
                            ps1,
                            lhsT=qslice(k, P, C),
                            rhs=qslice(k, P, C),
                            start=(k == 0),
                            stop=(k == KT - 1),
                        )
                st0 = sp.tile([P, C], bf16, tag="st0", name=f"st0_{c}")
                st1 = sp.tile([P, P], bf16, tag="st1", name=f"st1_{c}")
                nc.vector.tensor_mul(st0[:, 0:P], ps0[:, 0:P], mtile)
                nc.scalar.copy(st0[:, P:C], ps0[:, P:C])
                nc.vector.tensor_mul(st1, ps1, mtile)

                # out rows of this chunk: q @ S_{<c} + intra @ V
                ot = op_.tile([P, CSUB * D], bf16)
                po0 = ppo.tile([P, D], f32, tag="po", name=f"po0_{c}")
                first = True
                if c > 0:
                    for k in range(KT):
                        nc.tensor.matmul(
                            po0,
                            lhsT=qslice(k, 0, P),
                            rhs=S_k(k),
                            start=first,
                            stop=False,
                        )
                        first = False
                nc.tensor.matmul(
                    po0,
                    lhsT=st0[:, 0:P],
                    rhs=vtiles[CSUB * c],
                    start=first,
                    stop=True,
                )
                nc.scalar.copy(ot[:, 0:D], po0)
                nc.scalar.dma_start(
                    out=out[c0 : c0 + P, :], in_=ot[:, 0:D]
                )

                po1 = ppo.tile([P, D], f32, tag="po", name=f"po1_{c}")
                first = True
                if c > 0:
                    for k in range(KT):
                        nc.tensor.matmul(
                            po1,
                            lhsT=qslice(k, P, C),
                            rhs=S_k(k),
                            start=first,
                            stop=False,
                        )
                        first = False
                nc.tensor.matmul(
                    po1,
                    lhsT=st0[:, P:C],
                    rhs=vtiles[CSUB * c],
                    start=first,
                    stop=False,
                )
                nc.tensor.matmul(
                    po1,
                    lhsT=st1,
                    rhs=vtiles[CSUB * c + 1],
                    start=False,
                    stop=True,
                )
                # stores issue from Activation (which wrote ot): keeps the SP
                # queue a pure prefetch stream, and per-half so the final
                # store drains while po1 of the same chunk is still computing
                nc.scalar.copy(ot[:, D : 2 * D], po1)
                # the very last store is the kernel tail: SP sits idle there
                # and has a slightly shorter DGE chain than Activation
                st_eng = nc.sync if c == NCH - 1 else nc.scalar
                st_eng.dma_start(
                    out=out[c0 + P : c0 + C, :], in_=ot[:, D : 2 * D]
                )

                # state update: S[k] += qtn_c[:, k-tile].T @ V_chunk, folded
                # pairwise from one PSUM bank. (the state after the last
                # chunk is never read)
                if c == NCH - 1:
                    continue
                for j in range(KP):
                    pu = ppu.tile([P, 2 * D], f32)
                    for h in range(2):
                        k = 2 * j + h
                        for m in range(CSUB):
                            nc.tensor.matmul(
                                pu[:, h * D : (h + 1) * D],
                                lhsT=tnslice(k, m),
                                rhs=vtiles[CSUB * c + m],
                                start=(m == 0),
                                stop=(m == CSUB - 1),
                            )
                    # GPSIMD cannot touch PSUM: odd pairs bounce through an
                    # Activation-engine bf16 eviction, even pairs fold on DVE.
                    # Parity interleave: the two drains run in parallel, so pu
                    # slots free faster than PE produces pairs (no lockstep).
                    if c == 0:
                        if j % 2 == 0:
                            nc.vector.tensor_copy(Spairs[j], pu)
                        else:
                            nc.scalar.copy(Spairs[j], pu)
                    elif j % 2 == 0:
                        nc.vector.tensor_add(Spairs[j], Spairs[j], pu)
                    else:
                        tmp = tmp_.tile([P, 2 * D], bf16, tag="tm", name=f"tm{c}_{j}")
                        nc.scalar.copy(tmp, pu)
                        nc.gpsimd.tensor_add(Spairs[j], Spairs[j], tmp)

    nc.compile()
    return nc


def _get_compiled():
    global _COMPILED
    if _COMPILED is None:
        _COMPILED = _build()
    return _COMPILED


def _pack_core(sl):
    """sl: [T, NC_FEAT] bf16 QR slice for one core -> (qrt_p, qtn_p)."""
    # qrt_p[c, p, k*C+j] = sl[c*C+j, k*128+p]
    qrt_p = np.ascontiguousarray(
        sl.reshape(NCH, C, KT, P).transpose(0, 3, 2, 1).reshape(NCH, P, KT * C)
    )
    # k-major: qtn_p[c, p, k*C + m*P + n] = sl[c*C+m*128+p, k*128+n]
    qtn_p = np.ascontiguousarray(
        sl.reshape(NCH, CSUB, P, KT, P).transpose(0, 2, 3, 1, 4).reshape(
            NCH, P, KT * C
        )
    )
    return qrt_p, qtn_p


def kernel(Q, V, _want_results=False, **_unused):
    import ml_dtypes

    from concourse import bass_utils

    bf16 = ml_dtypes.bfloat16
    Q = np.asarray(Q, dtype=np.float32)
    V = np.asarray(V, dtype=np.float32)

    mask_np = _mask_host()
    # v_p[p, a*D+d] = V[0, 0, a*128+p, d]
    v_p = np.ascontiguousarray(
        V[0, 0].astype(bf16).reshape(TT, P, D).transpose(1, 0, 2).reshape(P, TT * D)
    )

    E = _rope_tables()

    def _prep(args):
        # per-core RoPE + bf16 cast + pack, threaded (numpy copies drop GIL)
        h, half = args
        sl_f32 = (
            Q[0, h, :, half * NC_FEAT : (half + 1) * NC_FEAT].view(np.complex64)
            * E[:, half * (NC_FEAT // 2) : (half + 1) * (NC_FEAT // 2)]
        ).view(np.float32)
        sl = sl_f32.astype(bf16)
        qrt_p, qtn_p = _pack_core(sl)
        return {"qrt": qrt_p, "qtn": qtn_p, "v": v_p, "mask": mask_np}

    from concurrent.futures import ThreadPoolExecutor

    jobs = [(h, half) for h in range(NH) for half in range(NSPLIT)]
    with ThreadPoolExecutor(max_workers=NCORES) as ex:
        in_maps = list(ex.map(_prep, jobs))

    nc = _get_compiled()
    res = bass_utils.run_bass_kernel_spmd(nc, in_maps, core_ids=list(range(NCORES)))

    out = np.empty((B, NH, T, D), dtype=np.float32)
    for h in range(NH):
        out[0, h] = res.results[2 * h]["out"].astype(np.float32) + res.results[
            2 * h + 1
        ]["out"].astype(np.float32)
    if _want_results:
        return out, res
    return out


if __name__ == "__main__":
    rng = np.random.default_rng(0)
    Q = (rng.standard_normal((B, NH, T, N)) * 0.02).astype(np.float32)
    V = rng.standard_normal((B, 1, T, D)).astype(np.float32)
    out = kernel(Q=Q, V=V)
    print("out", out.shape, out.dtype, float(np.abs(out).max()))


# revision 50
# speedup vs baseline: 1.0018x; 1.0018x over previous
"""Sparse attention (RoPE'd Q=K, strictly-causal unnormalized scores @ V).

  Q: (1, 4, 2048, 8192) f32   V: (1, 1, 2048, 256) f32
  out = tril(QR @ QR^T, -1) @ V   per head, V broadcast over heads.

Sharding: 8 cores = 4 heads x 2 halves of the N=8192 contraction dim.
The causal mask is elementwise, so masked-scores @ V is additive over
N-slices: each core computes a full (2048, 256) partial output from its
(2048, 4096) slice of QR; host sums the two halves per head.

Device algorithm (chunked linear attention, chunk C=256):
  out[t] = QR[t] @ S_{<chunk} + (intra-chunk causal part), where
  S = sum_s QR[s] (x) V[s] is an [N_c, D] state accumulated chunk by chunk.

v2 over the f32r baseline:
  - All matmul operands bf16 (rel err ~4e-3 vs 2e-2 budget): halves HBM
    traffic and lifts the f32r free-dim>=256 restriction, so the dead
    lower-left intra block is skipped (st1 computed only for its live
    128 columns). PE cost model: 1 cycle/row at any free size.
  - State kept as 16 pairs [128, 2*D] bf16; each update accumulates into
    one PSUM bank [128, 512] f32 and is folded with a single paired
    tensor_add, split DVE (first 9, matches Q@S consumption order) /
    GPSIMD (last 7).
  - PSUM->SBUF evictions (out rows, intra cross block) on the otherwise
    idle Activation engine.

Host does RoPE in f32, then packs bf16 so each chunk is a single large
DMA with 8 KiB contiguous descriptor runs per partition:
  qrt_p[c]  = [128, 32*256]  (SBUF layout: partition p=n%128, k-tile major)
  qtn_p[c]  = [128, 2*4096]  (partition p=t%128 within chunk)
  v_p       = [128, 16*256]
"""

import math

import numpy as np

THETA = 2.0**16
TWO_PI = 2.0 * math.pi

B, NH, T, N, D = 1, 4, 2048, 8192, 256
NSPLIT = 2
NCORES = NH * NSPLIT
NC_FEAT = N // NSPLIT  # 4096 features per core
P = 128
KT = NC_FEAT // P  # 32 n-tiles
KP = KT // 2  # 16 state pairs
TT = T // P  # 16 t-tiles
C = 256  # chunk length
NCH = T // C  # 8 chunks
CSUB = C // P  # 2 t-subtiles per chunk
# qrt DMA piece size in k-tiles: piece 0's matmul work covers piece 1's
# arrival, so 4 uniform pieces stream without quantization stalls
QPIECES = (8, 8, 8, 8)
VSPLIT = 4  # v subtiles loaded up front (covers chunks 0-2); rest deferred

_COMPILED = None
_ROPE_E = None


def _rope_tables():
    """cos/sin as one complex table; frequencies are pair-constant, so only
    even columns are needed. Input-independent -> cached across calls."""
    global _ROPE_E
    if _ROPE_E is None:
        idx = (np.floor(np.arange(N, dtype=np.float32) / 2.0) * 2.0).astype(
            np.float32
        )
        freqs = (1.0 / (THETA ** (idx / np.float32(N))) / np.float32(TWO_PI)).astype(
            np.float32
        )
        t = np.arange(T, dtype=np.float32)
        phases = t[:, None] * freqs[None, ::2]
        ang = np.float32(TWO_PI) * (phases % np.float32(1.0))
        E = np.empty((T, N // 2), np.complex64)
        E.real = np.cos(ang)
        E.imag = np.sin(ang)
        _ROPE_E = E
    return _ROPE_E


def _rope_host(Q):
    """(a+bi)(c+si) = (ac-bs) + (as+bc)i == the reference's interleaved
    rotate-pairs RoPE, one pass over Q viewed as complex64."""
    E = _rope_tables()
    QRc = Q.view(np.complex64) * E
    return QRc.view(np.float32)


def _mask_host():
    """mask[si, tj] = 1 if si < tj; shared by both diagonal intra blocks."""
    si = np.arange(P)[:, None]
    tj = np.arange(P)[None, :]
    return (si < tj).astype(np.float32)  # [128, 128]


def _build():
    import concourse.tile as tile
    from concourse import bacc, mybir

    nc = bacc.Bacc(
        "TRN2",
        target_bir_lowering=False,
        debug=False,
        enable_asserts=False,
        num_devices=NCORES,
    )
    f32 = mybir.dt.float32
    bf16 = mybir.dt.bfloat16

    qrt = nc.dram_tensor("qrt", [NCH, P, KT * C], bf16, kind="ExternalInput").ap()
    qtn = nc.dram_tensor("qtn", [NCH, P, KT * C], bf16, kind="ExternalInput").ap()
    v = nc.dram_tensor("v", [P, TT * D], bf16, kind="ExternalInput").ap()
    mask = nc.dram_tensor("mask", [P, P], f32, kind="ExternalInput").ap()
    out = nc.dram_tensor("out", [T, D], bf16, kind="ExternalOutput").ap()

    with tile.TileContext(nc) as tc:
        with (
            tc.tile_pool(name="qr", bufs=16) as qp,
            tc.tile_pool(name="qt", bufs=12) as tp,
            tc.tile_pool(name="vp", bufs=1) as vp,
            tc.tile_pool(name="mk", bufs=1) as mp,
            tc.tile_pool(name="st", bufs=KP) as stp,
            tc.tile_pool(name="sc", bufs=4) as sp,
            tc.tile_pool(name="ob", bufs=3) as op_,
            tc.tile_pool(name="tm", bufs=3) as tmp_,
            tc.tile_pool(name="p0", bufs=1, space="PSUM") as pp0,
            tc.tile_pool(name="po", bufs=2, space="PSUM") as ppo,
            tc.tile_pool(name="pu", bufs=5, space="PSUM") as ppu,
        ):
            vtiles = None
            mtile = None
            # state pair j holds S[2j] | S[2j+1], each [128, D]
            Spairs = [
                stp.tile([P, 2 * D], bf16, tag="S", name=f"S{j}") for j in range(KP)
            ]

            # warm the Activation func table during the startup DMA wait so
            # the implicit LoadActFuncSet is off the critical path
            warm = tmp_.tile([P, 1], f32, tag="wu", name="warm")
            nc.vector.memset(warm, 0.0)
            nc.scalar.copy(warm, warm)
            # burn the PE pstate ramp on garbage matmuls while the first qrt
            # piece is in flight: by the first real matmul the clock is at
            # 2.4GHz instead of spending chunk 0 at 0.65-1.2GHz
            wb = tmp_.tile([P, 2 * P], bf16, tag="wb", name="wb")
            nc.vector.memset(wb, 0.0)
            wpo = ppo.tile([P, 2 * P], f32, tag="po", name="warm_po")
            for i in range(15):
                nc.tensor.matmul(
                    wpo, lhsT=wb[:, 0:P], rhs=wb, start=(i == 0), stop=(i == 14)
                )

            def S_k(k):
                return Spairs[k // 2][:, (k % 2) * D : (k % 2) * D + D]

            for c in range(NCH):
                c0 = c * C
                pieces = QPIECES
                qh = []  # (first_ktile, tile)
                k0 = 0
                for u, nk in enumerate(pieces):
                    qhu = qp.tile([P, nk * C], bf16, tag="qr", name=f"q{c}_{u}")
                    nc.sync.dma_start(
                        out=qhu, in_=qrt[c][:, k0 * C : (k0 + nk) * C]
                    )
                    qh.append((k0, qhu))
                    k0 += nk
                    if c == 0 and u == 0:
                        # tiny; lands before the first st mask-mul needs it
                        mtile = mp.tile([P, P], f32)
                        nc.sync.dma_start(out=mtile, in_=mask)

                def qslice(k, lo, hi):
                    for k0, qhu in reversed(qh):
                        if k >= k0:
                            return qhu[:, (k - k0) * C + lo : (k - k0) * C + hi]
                    raise AssertionError

                if c == 0:
                    # v split: the early phase is bus-bound (qrt_0+qtn_0+qrt_1
                    # must land before chunk 1), so defer most of v past qrt_1
                    vt = vp.tile([P, TT * D], bf16)
                    nc.sync.dma_start(
                        out=vt[:, : VSPLIT * D], in_=v[:, : VSPLIT * D]
                    )
                    vtiles = [vt[:, a * D : (a + 1) * D] for a in range(TT)]
                if c == 1:
                    nc.sync.dma_start(
                        out=vt[:, VSPLIT * D :], in_=v[:, VSPLIT * D :]
                    )

                # qtn is packed k-major ([k, m, n] per partition row), so the
                # update can start after the first piece instead of the full
                # 2 MB (the early chunks are DMA-bandwidth-bound)
                tn_pieces = []  # (first_ktile, tile)
                if c < NCH - 1:
                    tk0 = 0
                    for nk in QPIECES:
                        tnp = tp.tile(
                            [P, nk * C], bf16, tag="tn", name=f"tn{c}_{tk0}"
                        )
                        nc.sync.dma_start(
                            out=tnp, in_=qtn[c][:, tk0 * C : (tk0 + nk) * C]
                        )
                        tn_pieces.append((tk0, tnp))
                        tk0 += nk

                def tnslice(k, m):
                    for tk0, tnp in reversed(tn_pieces):
                        if k >= tk0:
                            base = (k - tk0) * C + m * P
                            return tnp[:, base : base + P]
                    raise AssertionError

                # intra-chunk causal scores, [s, t] upper layout.
                # Block s0 x (t0|t1): [128, 256]; block s1 x t1: [128, 128]
                # (s1 x t0 is identically zero and skipped).
                pi_t = pp0.tile([P, C + P], f32, tag="ps", name=f"ps_{c}")
                ps0 = pi_t[:, 0:C]
                ps1 = pi_t[:, C : C + P]
                if c == 0:
                    # chunk 0 streams behind its own DMA: interleave both
                    # score groups per k so each arriving piece carries 2x
                    # the matmul work. Interleaved open accumulation groups
                    # must sit in DIFFERENT banks: borrow a po slot for ps1.
                    ps1 = ppo.tile([P, P], f32, tag="po", name="ps1_0")
                    for k in range(KT):
                        nc.tensor.matmul(
                            ps0,
                            lhsT=qslice(k, 0, P),
                            rhs=qslice(k, 0, C),
                            start=(k == 0),
                            stop=(k == KT - 1),
                            skip_group_check=True,
                        )
                        nc.tensor.matmul(
                            ps1,
                            lhsT=qslice(k, P, C),
                            rhs=qslice(k, P, C),
                            start=(k == 0),
                            stop=(k == KT - 1),
                            skip_group_check=True,
                        )
                else:
                    for k in range(KT):
                        nc.tensor.matmul(
                            ps0,
                            lhsT=qslice(k, 0, P),
                            rhs=qslice(k, 0, C),
                            start=(k == 0),
                            stop=(k == KT - 1),
                        )
                    for k in range(KT):
                        nc.tensor.matmul(
                            ps1,
                            lhsT=qslice(k, P, C),
                            rhs=qslice(k, P, C),
                            start=(k == 0),
                            stop=(k == KT - 1),
                        )
                st0 = sp.tile([P, C], bf16, tag="st0", name=f"st0_{c}")
                st1 = sp.tile([P, P], bf16, tag="st1", name=f"st1_{c}")
                nc.vector.tensor_mul(st0[:, 0:P], ps0[:, 0:P], mtile)
                nc.scalar.copy(st0[:, P:C], ps0[:, P:C])
                nc.vector.tensor_mul(st1, ps1, mtile)

                # out rows of this chunk: q @ S_{<c} + intra @ V
                ot = op_.tile([P, CSUB * D], bf16)
                po0 = ppo.tile([P, D], f32, tag="po", name=f"po0_{c}")
                first = True
                if c > 0:
                    for k in range(KT):
                        nc.tensor.matmul(
                            po0,
                            lhsT=qslice(k, 0, P),
                            rhs=S_k(k),
                            start=first,
                            stop=False,
                        )
                        first = False
                nc.tensor.matmul(
                    po0,
                    lhsT=st0[:, 0:P],
                    rhs=vtiles[CSUB * c],
                    start=first,
                    stop=True,
                )
                nc.scalar.copy(ot[:, 0:D], po0)
                nc.scalar.dma_start(
                    out=out[c0 : c0 + P, :], in_=ot[:, 0:D]
                )

                po1 = ppo.tile([P, D], f32, tag="po", name=f"po1_{c}")
                first = True
                if c > 0:
                    for k in range(KT):
                        nc.tensor.matmul(
                            po1,
                            lhsT=qslice(k, P, C),
                            rhs=S_k(k),
                            start=first,
                            stop=False,
                        )
                        first = False
                nc.tensor.matmul(
                    po1,
                    lhsT=st0[:, P:C],
                    rhs=vtiles[CSUB * c],
                    start=first,
                    stop=False,
                )
                nc.tensor.matmul(
                    po1,
                    lhsT=st1,
                    rhs=vtiles[CSUB * c + 1],
                    start=False,
                    stop=True,
                )
                # stores issue from Activation (which wrote ot): keeps the SP
                # queue a pure prefetch stream, and per-half so the final
                # store drains while po1 of the same chunk is still computing
                nc.scalar.copy(ot[:, D : 2 * D], po1)
                # the very last store is the kernel tail: SP sits idle there
                # and has a slightly shorter DGE chain than Activation
                st_eng = nc.sync if c == NCH - 1 else nc.scalar
                st_eng.dma_start(
                    out=out[c0 + P : c0 + C, :], in_=ot[:, D : 2 * D]
                )

                # state update: S[k] += qtn_c[:, k-tile].T @ V_chunk, folded
                # pairwise from one PSUM bank. (the state after the last
                # chunk is never read)
                if c == NCH - 1:
                    continue
                for j in range(KP):
                    pu = ppu.tile([P, 2 * D], f32)
                    for h in range(2):
                        k = 2 * j + h
                        for m in range(CSUB):
                            nc.tensor.matmul(
                                pu[:, h * D : (h + 1) * D],
                                lhsT=tnslice(k, m),
                                rhs=vtiles[CSUB * c + m],
                                start=(m == 0),
                                stop=(m == CSUB - 1),
                            )
                    # GPSIMD cannot touch PSUM: odd pairs bounce through an
                    # Activation-engine bf16 eviction, even pairs fold on DVE.
                    # Parity interleave: the two drains run in parallel, so pu
                    # slots free faster than PE produces pairs (no lockstep).
                    if c == 0:
                        if j % 2 == 0:
                            nc.vector.tensor_copy(Spairs[j], pu)
                        else:
                            nc.scalar.copy(Spairs[j], pu)
                    elif j % 2 == 0:
                        nc.vector.tensor_add(Spairs[j], Spairs[j], pu)
                    else:
                        tmp = tmp_.tile([P, 2 * D], bf16, tag="tm", name=f"tm{c}_{j}")
                        nc.scalar.copy(tmp, pu)
                        nc.gpsimd.tensor_add(Spairs[j], Spairs[j], tmp)

    nc.compile()
    return nc


def _get_compiled():
    global _COMPILED
    if _COMPILED is None:
        _COMPILED = _build()
    return _COMPILED


def _pack_core(sl):
    """sl: [T, NC_FEAT] bf16 QR slice for one core -> (qrt_p, qtn_p)."""
    # qrt_p[c, p, k*C+j] = sl[c*C+j, k*128+p]
    qrt_p = np.ascontiguousarray(
        sl.reshape(NCH, C, KT, P).transpose(0, 3, 2, 1).reshape(NCH, P, KT * C)
    )
    # k-major: qtn_p[c, p, k*C + m*P + n] = sl[c*C+m*128+p, k*128+n]
    qtn_p = np.ascontiguousarray(
        sl.reshape(NCH, CSUB, P, KT, P).transpose(0, 2, 3, 1, 4).reshape(
            NCH, P, KT * C
        )
    )
    return qrt_p, qtn_p


def kernel(Q, V, _want_results=False, **_unused):
    import ml_dtypes

    from concourse import bass_utils

    bf16 = ml_dtypes.bfloat16
    Q = np.asarray(Q, dtype=np.float32)
    V = np.asarray(V, dtype=np.float32)

    mask_np = _mask_host()
    # v_p[p, a*D+d] = V[0, 0, a*128+p, d]
    v_p = np.ascontiguousarray(
        V[0, 0].astype(bf16).reshape(TT, P, D).transpose(1, 0, 2).reshape(P, TT * D)
    )

    E = _rope_tables()

    def _prep(args):
        # per-core RoPE + bf16 cast + pack, threaded (numpy copies drop GIL)
        h, half = args
        sl_f32 = (
            Q[0, h, :, half * NC_FEAT : (half + 1) * NC_FEAT].view(np.complex64)
            * E[:, half * (NC_FEAT // 2) : (half + 1) * (NC_FEAT // 2)]
        ).view(np.float32)
        sl = sl_f32.astype(bf16)
        qrt_p, qtn_p = _pack_core(sl)
        return {"qrt": qrt_p, "qtn": qtn_p, "v": v_p, "mask": mask_np}

    from concurrent.futures import ThreadPoolExecutor

    jobs = [(h, half) for h in range(NH) for half in range(NSPLIT)]
    with ThreadPoolExecutor(max_workers=NCORES) as ex:
        in_maps = list(ex.map(_prep, jobs))

    nc = _get_compiled()
    res = bass_utils.run_bass_kernel_spmd(nc, in_maps, core_ids=list(range(NCORES)))

    out = np.empty((B, NH, T, D), dtype=np.float32)
    for h in range(NH):
        out[0, h] = res.results[2 * h]["out"].astype(np.float32) + res.results[
            2 * h + 1
        ]["out"].astype(np.float32)
    if _want_results:
        return out, res
    return out


if __name__ == "__main__":
    rng = np.random.default_rng(0)
    Q = (rng.standard_normal((B, NH, T, N)) * 0.02).astype(np.float32)
    V = rng.standard_normal((B, 1, T, D)).astype(np.float32)
    out = kernel(Q=Q, V=V)
    print("out", out.shape, out.dtype, float(np.abs(out).max()))
